# revision 22
# baseline (speedup 1.0000x reference)
"""BA3TGCN2 Trainium2 kernel: dst-sharded GCN gather/segment-sum + GRU gate fusion.

Math (H0 == 0 makes the R gate dead and linearizes the layers):
  out[b,n,:] = sum_p ws[p] * sigmoid(-(Ahat x_p Uz + bz)) * tanh(Ahat x_p Uh + bh)
  Uz = Wcz @ Wlz[:COUT], bz = bcz @ Wlz[:COUT] + blz   (same for h with Wch/Wlh)
  ws = softmax(attention) (second half scaled by TRAIN_OR_PREDICT=1)

Sharding: dst nodes balanced-binned into 80 blocks of <=128 slots, 10 blocks per
core; every core handles ALL 16 batches for its ~1250 dst nodes (~E/8 edges).
Per-edge gather row: 4KB = 16 batches x 16 periods x 8 cin, bf16, fetched with
batched indirect DMA (G chunks / 640 rows per SWDGE call).
Per 128-edge chunk: S[e,d] = (iota==dstrel)*norm one-hot scatter matrix, then
ytb[d, feat] += S^T @ gathered (TensorE segment-sum, PSUM f32, 4 banks).
Per block: copy->SBUF bf16, 16 TensorE transposes -> per-batch (p*8+c, dst)
tiles, 8 gate matmuls vs fused U, sigmoid/tanh on ACT, product on DVE,
period-weighted sum via wsum matmul, stage, one DMA out per batch.
"""

import heapq
import os

import numpy as np
import ml_dtypes

import concourse.bass as bass
import concourse.bacc as bacc
from concourse._compat import get_trn_type
import concourse.mybir as mybir
import concourse.tile as tile
from concourse import library_config
from concourse.bass_utils import run_bass_kernel_spmd

BF16 = ml_dtypes.bfloat16
FP8E3 = ml_dtypes.float8_e3m4

B, N, CIN, COUT, P2 = 16, 10000, 8, 32, 16
E = 160000
NCORES = 8
NBLK = 10                    # dst blocks per core, 128 slots each
NBINS = NCORES * NBLK        # 80
FEAT = B * P2 * CIN          # 2048 = full-batch feature row per node
G = 1                        # 128-edge chunks per indirect-gather call
TRAIN_OR_PREDICT = 1.0

LAST_RESULT = None           # BassKernelResults of last run (for test.py)


def _softmax(x):
    e = np.exp(x - np.max(x))
    return e / e.sum()


def prep_host(X, edge_index, edge_weight, attention,
              Wcz, bcz, Wlz, blz, Wcr, bcr, Wlr, blr, Wch, bch, Wlh, blh):
    X = np.asarray(X, np.float32)
    src = np.asarray(edge_index[0], np.int64)
    dst = np.asarray(edge_index[1], np.int64)
    w = np.asarray(edge_weight, np.float32)

    # gcn_norm with self loops
    loop = np.arange(N, dtype=np.int64)
    src = np.concatenate([src, loop])
    dst = np.concatenate([dst, loop])
    w = np.concatenate([w, np.ones(N, np.float32)])
    deg = np.bincount(dst, weights=w, minlength=N).astype(np.float32)
    dinv = np.where(deg > 0, deg.astype(np.float64) ** -0.5, 0.0).astype(np.float32)
    norm = (dinv[src] * w * dinv[dst]).astype(np.float32)

    # balance dst nodes into NBINS bins (<=128 each) with ~equal edge counts
    cnt = np.bincount(dst, minlength=N)
    order = np.argsort(-cnt, kind="stable")
    bin_of = np.empty(N, np.int32)
    slot_of = np.empty(N, np.int32)
    ndst = np.zeros(NBINS, np.int32)
    tot = np.zeros(NBINS, np.int64)
    hp = [(0, b) for b in range(NBINS)]
    heapq.heapify(hp)
    for nid in order:
        while True:
            t, b = heapq.heappop(hp)
            if ndst[b] < 128:
                break
        bin_of[nid] = b
        slot_of[nid] = ndst[b]
        ndst[b] += 1
        tot[b] += cnt[nid]
        if ndst[b] < 128:
            heapq.heappush(hp, (int(t + cnt[nid]), b))
    NCB = int(-(-int(tot.max()) // 128))     # chunks per block (uniform)
    NCH = NBLK * NCB

    # per-core edge streams: (core, 128 lanes, NCH cols), col = blk*NCB + chunk
    ebin = bin_of[dst]
    # sort by (bin, src): ascending src within each bin gives the 4KB gather
    # reads an ascending-address sweep over HBM (better row locality)
    eorder = np.lexsort((src, ebin))
    bcnt = np.bincount(ebin, minlength=NBINS)
    boff = np.concatenate([[0], np.cumsum(bcnt)])
    sb = ebin[eorder]
    pos = np.arange(sb.size) - boff[sb]
    lane = (pos % 128).astype(np.int64)
    col = (sb % NBLK).astype(np.int64) * NCB + pos // 128
    core = sb // NBLK
    # full-batch node feature table, row n = [b(16) x p(16) x cin(8)] fp8-e3m4,
    # per-node scaled into the e3m4 sweet spot; 1/scale folded into edge norms
    xt32 = np.ascontiguousarray(X.transpose(1, 0, 3, 2).reshape(N, FEAT))
    rowmax = np.abs(xt32).max(axis=1)
    xscale = (12.0 / np.maximum(rowmax, 1e-9)).astype(np.float32)
    xtab = (xt32 * xscale[:, None]).astype(FP8E3)

    gidx = np.zeros((NCORES, 128, NCH), np.int32)
    dstrel = np.zeros((NCORES, 128, NCH), np.float32)
    normt = np.zeros((NCORES, 128, NCH), np.float32)
    gidx[core, lane, col] = src[eorder]
    dstrel[core, lane, col] = slot_of[dst[eorder]]
    normt[core, lane, col] = norm[eorder] / xscale[src[eorder]]

    # dma_gather index layout: per call bi (G chunks), gathered row i=j*128+p
    # reads gidx[p, bi*G+j]; idxs wrap 16 partitions: idxs16[i%16, i//16]
    NCALLS = NCH // G
    CW = G * 128 // 16           # idx columns per call
    gidx16 = np.zeros((NCORES, 128, NCALLS * CW), np.int16)
    for bi in range(NCALLS):
        # unwrapped[i = j*128+p] = gidx[core, p, bi*G+j]
        unwrapped = np.ascontiguousarray(
            gidx[:, :, bi * G:(bi + 1) * G].transpose(0, 2, 1)  # (core, j, p)
        ).reshape(NCORES, G * 128).astype(np.int16)
        gidx16[:, :16, bi * CW:(bi + 1) * CW] = unwrapped.reshape(
            NCORES, CW, 16).transpose(0, 2, 1)


    # fused weights / biases / period weights
    Uz = np.asarray(Wcz, np.float32) @ np.asarray(Wlz, np.float32)[:COUT]
    Uh = np.asarray(Wch, np.float32) @ np.asarray(Wlh, np.float32)[:COUT]
    bz = np.asarray(bcz, np.float32) @ np.asarray(Wlz, np.float32)[:COUT] + np.asarray(blz, np.float32)
    bh = np.asarray(bch, np.float32) @ np.asarray(Wlh, np.float32)[:COUT] + np.asarray(blh, np.float32)
    probs = _softmax(np.asarray(attention, np.float32))
    ws = np.concatenate([probs[:P2 // 2], probs[P2 // 2:] * TRAIN_OR_PREDICT])

    # transform lhsT tiles: ubig[(p*8+cin), (g*4+grp)*128 + pl*32 + s] = (p==grp*4+pl)*U_g[cin,s]
    ubig = np.zeros((128, 2 * 4 * 128), np.float32)
    for g, U in enumerate((Uz, Uh)):
        for grp in range(4):
            for pl in range(4):
                p = grp * 4 + pl
                ubig[p * 8:(p + 1) * 8, (g * 4 + grp) * 128 + pl * 32:(g * 4 + grp) * 128 + (pl + 1) * 32] = U
    # weighted period-sum lhsT: wsum[(pl*32+s), grp*32+o] = ws[grp*4+pl]*(s==o)
    wsum = np.zeros((128, 4 * 32), np.float32)
    for grp in range(4):
        for pl in range(4):
            for s in range(32):
                wsum[pl * 32 + s, grp * 32 + s] = ws[grp * 4 + pl]
    biasz = np.repeat(-bz[None, :], 4, 0).reshape(128, 1).astype(np.float32)
    biash = np.repeat(bh[None, :], 4, 0).reshape(128, 1).astype(np.float32)

    iota = np.tile(np.arange(128, dtype=np.float32), (128, 1))
    ident = np.eye(128, dtype=np.float32)

    shared = dict(
        xtab=xtab,
        ubig=ubig.astype(BF16),
        wsum=wsum.astype(BF16),
        biasz=biasz,
        biash=biash,
        iota=iota.astype(BF16),
        ident=ident.astype(BF16),
    )
    percore = [dict(gidx=np.ascontiguousarray(gidx[c]),
                    dstrel=np.ascontiguousarray(dstrel[c]),
                    normt=np.ascontiguousarray(normt[c]))
               for c in range(NCORES)]
    unperm = dict(core_of=bin_of // NBLK,
                  col_of=(bin_of % NBLK) * 128 + slot_of)
    return shared, percore, unperm, NCB


def build_bass(NCB):
    NCH = NBLK * NCB
    f32 = mybir.dt.float32
    bf16 = mybir.dt.bfloat16
    i32 = mybir.dt.int32
    fp8e3 = mybir.dt.float8e3
    Alu = mybir.AluOpType
    Act = mybir.ActivationFunctionType
    NCALLS = NCH // G
    CW = G * 128 // 16

    nc = bacc.Bacc(get_trn_type() or "TRN2")
    xtab_d = nc.dram_tensor("xtab", (N, FEAT), fp8e3, kind="ExternalInput")
    gidx_d = nc.dram_tensor("gidx", (128, NCH), i32, kind="ExternalInput")
    dstrel_d = nc.dram_tensor("dstrel", (128, NCH), f32, kind="ExternalInput")
    normt_d = nc.dram_tensor("normt", (128, NCH), f32, kind="ExternalInput")
    ubig_d = nc.dram_tensor("ubig", (128, 1024), bf16, kind="ExternalInput")
    wsum_d = nc.dram_tensor("wsum", (128, 128), bf16, kind="ExternalInput")
    biasz_d = nc.dram_tensor("biasz", (128, 1), f32, kind="ExternalInput")
    biash_d = nc.dram_tensor("biash", (128, 1), f32, kind="ExternalInput")
    iota_d = nc.dram_tensor("iota", (128, 128), bf16, kind="ExternalInput")
    ident_d = nc.dram_tensor("ident", (128, 128), bf16, kind="ExternalInput")
    out_d = nc.dram_tensor("out", (B, 32, NBLK * 128), f32, kind="ExternalOutput")

    with tile.TileContext(nc) as tc:
        with tc.tile_pool(name="const", bufs=1) as cpool, \
             tc.tile_pool(name="gp", bufs=4) as gpool, \
             tc.tile_pool(name="sp", bufs=8) as spool, \
             tc.tile_pool(name="ysb", bufs=2) as ysbpool, \
             tc.tile_pool(name="yt", bufs=2) as ytpool, \
             tc.tile_pool(name="wk", bufs=2) as wkpool, \
             tc.tile_pool(name="st", bufs=1) as stpool, \
             tc.tile_pool(name="py", bufs=1, space="PSUM") as pypool, \
             tc.tile_pool(name="pt", bufs=1, space="PSUM") as ptpool, \
             tc.tile_pool(name="pg", bufs=1, space="PSUM") as pgpool, \
             tc.tile_pool(name="po", bufs=1, space="PSUM") as popool:

            def cload(dram, shape, dtype, name):
                t = cpool.tile(shape, dtype, name=name, tag=name)
                nc.sync.dma_start(t[:], dram[:])
                return t

            gidx_sb = cload(gidx_d, [128, NCH], i32, "gidx_sb")
            dstrel_sb = cload(dstrel_d, [128, NCH], f32, "dstrel_sb")
            norm_sb = cload(normt_d, [128, NCH], f32, "norm_sb")
            ubig_sb = cload(ubig_d, [128, 1024], bf16, "ubig_sb")
            wsum_sb = cload(wsum_d, [128, 128], bf16, "wsum_sb")
            biasz_sb = cload(biasz_d, [128, 1], f32, "biasz_sb")
            biash_sb = cload(biash_d, [128, 1], f32, "biash_sb")
            iota_sb = cload(iota_d, [128, 128], bf16, "iota_sb")
            ident_sb = cload(ident_d, [128, 128], bf16, "ident_sb")

            stage = [stpool.tile([32, NBLK * 128], f32, name=f"stage{b}", tag=f"stage{b}")
                     for b in range(B)]

            gts = {}

            def gather(bi):
                gt = gpool.tile([128, G * FEAT], fp8e3, tag="gt", name="gt")
                nc.gpsimd.indirect_dma_start(
                    out=gt[:],
                    out_offset=None,
                    in_=xtab_d[:, :],
                    in_offset=bass.IndirectOffsetOnAxis(
                        ap=gidx_sb[:, bi * G:(bi + 1) * G], axis=0),
                )
                gts[bi] = gt
                gather.nxt = bi + 1
            gather.nxt = 0

            def gate(blk, b, ysbT):
                rhs = ysbT[:, b * 128:(b + 1) * 128]
                az = pgpool.tile([128, 512], f32, tag="az", name="az")
                ah = pgpool.tile([128, 512], f32, tag="ah", name="ah")
                for grp in range(4):
                    nc.tensor.matmul(
                        az[:, grp * 128:(grp + 1) * 128],
                        lhsT=ubig_sb[:, grp * 128:(grp + 1) * 128],
                        rhs=rhs, start=True, stop=True)
                    nc.tensor.matmul(
                        ah[:, grp * 128:(grp + 1) * 128],
                        lhsT=ubig_sb[:, (4 + grp) * 128:(5 + grp) * 128],
                        rhs=rhs, start=True, stop=True)
                zp = wkpool.tile([128, 512], bf16, tag="zp", name="zp")
                tp2 = wkpool.tile([128, 512], bf16, tag="tp2", name="tp2")
                nc.scalar.activation(zp[:], az[:], Act.Sigmoid,
                                     bias=biasz_sb[:, :1], scale=-1.0)
                nc.scalar.activation(tp2[:], ah[:], Act.Tanh,
                                     bias=biash_sb[:, :1], scale=1.0)
                cc = wkpool.tile([128, 512], bf16, tag="cc", name="cc")
                nc.vector.tensor_tensor(cc[:], zp[:], tp2[:], op=Alu.mult)
                outp = popool.tile([32, 128], f32, tag="outp", name="outp")
                for grp in range(4):
                    nc.tensor.matmul(
                        outp[:],
                        lhsT=wsum_sb[:, grp * 32:(grp + 1) * 32],
                        rhs=cc[:, grp * 128:(grp + 1) * 128],
                        start=(grp == 0), stop=(grp == 3))
                nc.vector.tensor_copy(stage[b][:, blk * 128:(blk + 1) * 128], outp[:])

            # gate pipelines of block k-1 are interleaved between segment-sum
            # chunks of block k so ACT latency hides under chunk matmuls
            pending = []
            for blk in range(NBLK):
                ytb = pypool.tile([128, FEAT], f32, tag="ytb", name="ytb")
                for j in range(NCB):
                    c = blk * NCB + j
                    bi, sl = divmod(c, G)
                    while gather.nxt <= min(bi + 3, NCALLS - 1):
                        gather(gather.nxt)
                    gt = gts[bi]
                    S = spool.tile([128, 128], bf16, tag="S", name="S")
                    nc.vector.tensor_scalar(
                        S[:], iota_sb[:],
                        dstrel_sb[:, c:c + 1], norm_sb[:, c:c + 1],
                        Alu.is_equal, Alu.mult,
                    )
                    for q in range(4):
                        nc.tensor.matmul(
                            ytb[:, q * 512:(q + 1) * 512],
                            lhsT=S[:],
                            rhs=gt[:, sl * FEAT + q * 512: sl * FEAT + (q + 1) * 512],
                            start=(j == 0), stop=(j == NCB - 1),
                        )
                    if pending:
                        pending.pop(0)()
                while pending:
                    pending.pop(0)()

                ysb = ysbpool.tile([128, FEAT], bf16, tag="ysb", name="ysb")
                nc.vector.tensor_copy(ysb[:], ytb[:])
                ysbT = ytpool.tile([128, FEAT], bf16, tag="ysbT", name="ysbT")
                for q in range(4):
                    tp = ptpool.tile([128, 512], bf16, tag="tp", name="tp")
                    for k in range(4):
                        fs = q * 4 + k
                        nc.tensor.transpose(
                            tp[:, k * 128:(k + 1) * 128],
                            ysb[:, fs * 128:(fs + 1) * 128], ident_sb[:])
                    nc.vector.tensor_copy(ysbT[:, q * 512:(q + 1) * 512], tp[:])

                pending = [
                    (lambda blk=blk, b=b, t=ysbT: gate(blk, b, t))
                    for b in range(B)
                ]
            while pending:
                pending.pop(0)()

            for b in range(B):
                nc.sync.dma_start(out_d[b], stage[b][:])

    nc.compile()
    return nc


def kernel(**inputs):
    global LAST_RESULT
    shared, percore, unperm, NCB = prep_host(**inputs)
    nc = build_bass(NCB)
    in_maps = []
    for c in range(NCORES):
        m = dict(shared)
        m.update(percore[c])
        in_maps.append(m)
    res = run_bass_kernel_spmd(nc, in_maps, core_ids=list(range(NCORES)),
                               trace=os.environ.get("BASS_TRACE") == "1")
    LAST_RESULT = res
    out = np.empty((B, N, COUT), np.float32)
    core_of, col_of = unperm["core_of"], unperm["col_of"]
    for c in range(NCORES):
        r = res.results[c]["out"]  # (B, 32, NBLK*128)
        ids = np.where(core_of == c)[0]
        out[:, ids, :] = r[:, :, col_of[ids]].transpose(0, 2, 1)
    return out


# revision 23
# speedup vs baseline: 1.1118x; 1.1118x over previous
"""BA3TGCN2 Trainium2 kernel: dst-sharded GCN gather/segment-sum + GRU gate fusion.

Math (H0 == 0 makes the R gate dead and linearizes the layers):
  out[b,n,:] = sum_p ws[p] * sigmoid(-(Ahat x_p Uz + bz)) * tanh(Ahat x_p Uh + bh)
  Uz = Wcz @ Wlz[:COUT], bz = bcz @ Wlz[:COUT] + blz   (same for h with Wch/Wlh)
  ws = softmax(attention) (second half scaled by TRAIN_OR_PREDICT=1)

Sharding: dst nodes balanced-binned into 80 blocks of <=128 slots, 10 blocks per
core; every core handles ALL 16 batches for its ~1250 dst nodes (~E/8 edges).
Per-edge gather row: 4KB = 16 batches x 16 periods x 8 cin, bf16, fetched with
batched indirect DMA (G chunks / 640 rows per SWDGE call).
Per 128-edge chunk: S[e,d] = (iota==dstrel)*norm one-hot scatter matrix, then
ytb[d, feat] += S^T @ gathered (TensorE segment-sum, PSUM f32, 4 banks).
Per block: copy->SBUF bf16, 16 TensorE transposes -> per-batch (p*8+c, dst)
tiles, 8 gate matmuls vs fused U, sigmoid/tanh on ACT, product on DVE,
period-weighted sum via wsum matmul, stage, one DMA out per batch.
"""

import heapq
import os

import numpy as np
import ml_dtypes

import concourse.bass as bass
import concourse.bacc as bacc
from concourse._compat import get_trn_type
import concourse.mybir as mybir
import concourse.tile as tile
from concourse import library_config
from concourse.bass_utils import run_bass_kernel_spmd

BF16 = ml_dtypes.bfloat16
FP8E3 = ml_dtypes.float8_e3m4

B, N, CIN, COUT, P2 = 16, 10000, 8, 32, 16
E = 160000
NCORES = 8
NBLK = 10                    # dst blocks per core, 128 slots each
NBINS = NCORES * NBLK        # 80
FEAT = B * P2 * CIN          # 2048 = full-batch feature row per node
G = 1                        # 128-edge chunks per indirect-gather call
TRAIN_OR_PREDICT = 1.0

LAST_RESULT = None           # BassKernelResults of last run (for test.py)


def _softmax(x):
    e = np.exp(x - np.max(x))
    return e / e.sum()


def prep_host(X, edge_index, edge_weight, attention,
              Wcz, bcz, Wlz, blz, Wcr, bcr, Wlr, blr, Wch, bch, Wlh, blh):
    X = np.asarray(X, np.float32)
    src = np.asarray(edge_index[0], np.int64)
    dst = np.asarray(edge_index[1], np.int64)
    w = np.asarray(edge_weight, np.float32)

    # gcn_norm with self loops
    loop = np.arange(N, dtype=np.int64)
    src = np.concatenate([src, loop])
    dst = np.concatenate([dst, loop])
    w = np.concatenate([w, np.ones(N, np.float32)])
    deg = np.bincount(dst, weights=w, minlength=N).astype(np.float32)
    dinv = np.where(deg > 0, deg.astype(np.float64) ** -0.5, 0.0).astype(np.float32)
    norm = (dinv[src] * w * dinv[dst]).astype(np.float32)

    # balance dst nodes into NBINS bins (<=128 each) with ~equal edge counts
    cnt = np.bincount(dst, minlength=N)
    order = np.argsort(-cnt, kind="stable")
    bin_of = np.empty(N, np.int32)
    slot_of = np.empty(N, np.int32)
    ndst = np.zeros(NBINS, np.int32)
    tot = np.zeros(NBINS, np.int64)
    hp = [(0, b) for b in range(NBINS)]
    heapq.heapify(hp)
    for nid in order:
        while True:
            t, b = heapq.heappop(hp)
            if ndst[b] < 128:
                break
        bin_of[nid] = b
        slot_of[nid] = ndst[b]
        ndst[b] += 1
        tot[b] += cnt[nid]
        if ndst[b] < 128:
            heapq.heappush(hp, (int(t + cnt[nid]), b))
    NCB = int(-(-int(tot.max()) // 128))     # chunks per block (uniform)
    NCH = NBLK * NCB

    # per-core edge streams: (core, 128 lanes, NCH cols), col = blk*NCB + chunk
    ebin = bin_of[dst]
    # sort by (bin, src): ascending src within each bin gives the 4KB gather
    # reads an ascending-address sweep over HBM (better row locality)
    eorder = np.lexsort((src, ebin))
    bcnt = np.bincount(ebin, minlength=NBINS)
    boff = np.concatenate([[0], np.cumsum(bcnt)])
    sb = ebin[eorder]
    pos = np.arange(sb.size) - boff[sb]
    lane = (pos % 128).astype(np.int64)
    col = (sb % NBLK).astype(np.int64) * NCB + pos // 128
    core = sb // NBLK
    # full-batch node feature table, row n = [b(16) x p(16) x cin(8)] fp8-e3m4,
    # per-node scaled into the e3m4 sweet spot; 1/scale folded into edge norms
    xt32 = np.ascontiguousarray(X.transpose(1, 0, 3, 2).reshape(N, FEAT))
    rowmax = np.abs(xt32).max(axis=1)
    xscale = (12.0 / np.maximum(rowmax, 1e-9)).astype(np.float32)
    xtab = (xt32 * xscale[:, None]).astype(FP8E3)

    gidx = np.zeros((NCORES, 128, NCH), np.int32)
    dstrel = np.zeros((NCORES, 128, NCH), np.float32)
    normt = np.zeros((NCORES, 128, NCH), np.float32)
    gidx[core, lane, col] = src[eorder]
    dstrel[core, lane, col] = slot_of[dst[eorder]]
    normt[core, lane, col] = norm[eorder] / xscale[src[eorder]]

    # dma_gather index layout: per call bi (G chunks), gathered row i=j*128+p
    # reads gidx[p, bi*G+j]; idxs wrap 16 partitions: idxs16[i%16, i//16]
    NCALLS = NCH // G
    CW = G * 128 // 16           # idx columns per call
    gidx16 = np.zeros((NCORES, 128, NCALLS * CW), np.int16)
    for bi in range(NCALLS):
        # unwrapped[i = j*128+p] = gidx[core, p, bi*G+j]
        unwrapped = np.ascontiguousarray(
            gidx[:, :, bi * G:(bi + 1) * G].transpose(0, 2, 1)  # (core, j, p)
        ).reshape(NCORES, G * 128).astype(np.int16)
        gidx16[:, :16, bi * CW:(bi + 1) * CW] = unwrapped.reshape(
            NCORES, CW, 16).transpose(0, 2, 1)


    # fused weights / biases / period weights
    Uz = np.asarray(Wcz, np.float32) @ np.asarray(Wlz, np.float32)[:COUT]
    Uh = np.asarray(Wch, np.float32) @ np.asarray(Wlh, np.float32)[:COUT]
    bz = np.asarray(bcz, np.float32) @ np.asarray(Wlz, np.float32)[:COUT] + np.asarray(blz, np.float32)
    bh = np.asarray(bch, np.float32) @ np.asarray(Wlh, np.float32)[:COUT] + np.asarray(blh, np.float32)
    probs = _softmax(np.asarray(attention, np.float32))
    ws = np.concatenate([probs[:P2 // 2], probs[P2 // 2:] * TRAIN_OR_PREDICT])

    # transform lhsT tiles: ubig[(p*8+cin), (g*4+grp)*128 + pl*32 + s] = (p==grp*4+pl)*U_g[cin,s]
    ubig = np.zeros((128, 2 * 4 * 128), np.float32)
    for g, U in enumerate((Uz, Uh)):
        for grp in range(4):
            for pl in range(4):
                p = grp * 4 + pl
                ubig[p * 8:(p + 1) * 8, (g * 4 + grp) * 128 + pl * 32:(g * 4 + grp) * 128 + (pl + 1) * 32] = U
    # weighted period-sum lhsT: wsum[(pl*32+s), grp*32+o] = ws[grp*4+pl]*(s==o)
    wsum = np.zeros((128, 4 * 32), np.float32)
    for grp in range(4):
        for pl in range(4):
            for s in range(32):
                wsum[pl * 32 + s, grp * 32 + s] = ws[grp * 4 + pl]
    biasz = np.repeat(-bz[None, :], 4, 0).reshape(128, 1).astype(np.float32)
    biash = np.repeat(bh[None, :], 4, 0).reshape(128, 1).astype(np.float32)

    iota = np.tile(np.arange(128, dtype=np.float32), (128, 1))
    ident = np.eye(128, dtype=np.float32)

    shared = dict(
        xtab=xtab,
        ubig=ubig.astype(BF16),
        wsum=wsum.astype(BF16),
        biasz=biasz,
        biash=biash,
        iota=iota.astype(BF16),
        ident=ident.astype(BF16),
    )
    percore = [dict(gidx=np.ascontiguousarray(gidx[c]),
                    dstrel=np.ascontiguousarray(dstrel[c]),
                    normt=np.ascontiguousarray(normt[c]))
               for c in range(NCORES)]
    unperm = dict(core_of=bin_of // NBLK,
                  col_of=(bin_of % NBLK) * 128 + slot_of)
    return shared, percore, unperm, NCB


def build_bass(NCB):
    NCH = NBLK * NCB
    f32 = mybir.dt.float32
    bf16 = mybir.dt.bfloat16
    i32 = mybir.dt.int32
    fp8e3 = mybir.dt.float8e3
    Alu = mybir.AluOpType
    Act = mybir.ActivationFunctionType
    NCALLS = NCH // G
    CW = G * 128 // 16

    nc = bacc.Bacc(get_trn_type() or "TRN2")
    xtab_d = nc.dram_tensor("xtab", (N, FEAT), fp8e3, kind="ExternalInput")
    gidx_d = nc.dram_tensor("gidx", (128, NCH), i32, kind="ExternalInput")
    dstrel_d = nc.dram_tensor("dstrel", (128, NCH), f32, kind="ExternalInput")
    normt_d = nc.dram_tensor("normt", (128, NCH), f32, kind="ExternalInput")
    ubig_d = nc.dram_tensor("ubig", (128, 1024), bf16, kind="ExternalInput")
    wsum_d = nc.dram_tensor("wsum", (128, 128), bf16, kind="ExternalInput")
    biasz_d = nc.dram_tensor("biasz", (128, 1), f32, kind="ExternalInput")
    biash_d = nc.dram_tensor("biash", (128, 1), f32, kind="ExternalInput")
    iota_d = nc.dram_tensor("iota", (128, 128), bf16, kind="ExternalInput")
    ident_d = nc.dram_tensor("ident", (128, 128), bf16, kind="ExternalInput")
    out_d = nc.dram_tensor("out", (B, 32, NBLK * 128), f32, kind="ExternalOutput")

    with tile.TileContext(nc) as tc:
        with tc.tile_pool(name="const", bufs=1) as cpool, \
             tc.tile_pool(name="gp", bufs=4) as gpool, \
             tc.tile_pool(name="sp", bufs=8) as spool, \
             tc.tile_pool(name="ysb", bufs=2) as ysbpool, \
             tc.tile_pool(name="yt", bufs=2) as ytpool, \
             tc.tile_pool(name="wk", bufs=2) as wkpool, \
             tc.tile_pool(name="st", bufs=1) as stpool, \
             tc.tile_pool(name="py", bufs=1, space="PSUM") as pypool, \
             tc.tile_pool(name="pt", bufs=1, space="PSUM") as ptpool, \
             tc.tile_pool(name="pg", bufs=1, space="PSUM") as pgpool, \
             tc.tile_pool(name="po", bufs=1, space="PSUM") as popool:

            def cload(dram, shape, dtype, name):
                t = cpool.tile(shape, dtype, name=name, tag=name)
                nc.sync.dma_start(t[:], dram[:])
                return t

            gidx_sb = cload(gidx_d, [128, NCH], i32, "gidx_sb")
            dstrel_sb = cload(dstrel_d, [128, NCH], f32, "dstrel_sb")
            norm_sb = cload(normt_d, [128, NCH], f32, "norm_sb")
            ubig_sb = cload(ubig_d, [128, 1024], bf16, "ubig_sb")
            wsum_sb = cload(wsum_d, [128, 128], bf16, "wsum_sb")
            biasz_sb = cload(biasz_d, [128, 1], f32, "biasz_sb")
            biash_sb = cload(biash_d, [128, 1], f32, "biash_sb")
            iota_sb = cload(iota_d, [128, 128], bf16, "iota_sb")
            ident_sb = cload(ident_d, [128, 128], bf16, "ident_sb")

            stage = [stpool.tile([32, NBLK * 128], f32, name=f"stage{b}", tag=f"stage{b}")
                     for b in range(B)]

            gts = {}

            def gather(bi):
                gt = gpool.tile([128, G * FEAT], bf16, tag="gt", name="gt")
                nc.gpsimd.indirect_dma_start(
                    out=gt[:],
                    out_offset=None,
                    in_=xtab_d[:, :],
                    in_offset=bass.IndirectOffsetOnAxis(
                        ap=gidx_sb[:, bi * G:(bi + 1) * G], axis=0),
                )
                gts[bi] = gt
                gather.nxt = bi + 1
            gather.nxt = 0

            def gate(blk, b, ysbT):
                rhs = ysbT[:, b * 128:(b + 1) * 128]
                az = pgpool.tile([128, 512], f32, tag="az", name="az")
                ah = pgpool.tile([128, 512], f32, tag="ah", name="ah")
                for grp in range(4):
                    nc.tensor.matmul(
                        az[:, grp * 128:(grp + 1) * 128],
                        lhsT=ubig_sb[:, grp * 128:(grp + 1) * 128],
                        rhs=rhs, start=True, stop=True)
                    nc.tensor.matmul(
                        ah[:, grp * 128:(grp + 1) * 128],
                        lhsT=ubig_sb[:, (4 + grp) * 128:(5 + grp) * 128],
                        rhs=rhs, start=True, stop=True)
                zp = wkpool.tile([128, 512], bf16, tag="zp", name="zp")
                tp2 = wkpool.tile([128, 512], bf16, tag="tp2", name="tp2")
                nc.scalar.activation(zp[:], az[:], Act.Sigmoid,
                                     bias=biasz_sb[:, :1], scale=-1.0)
                nc.scalar.activation(tp2[:], ah[:], Act.Tanh,
                                     bias=biash_sb[:, :1], scale=1.0)
                cc = wkpool.tile([128, 512], bf16, tag="cc", name="cc")
                nc.vector.tensor_tensor(cc[:], zp[:], tp2[:], op=Alu.mult)
                outp = popool.tile([32, 128], f32, tag="outp", name="outp")
                for grp in range(4):
                    nc.tensor.matmul(
                        outp[:],
                        lhsT=wsum_sb[:, grp * 32:(grp + 1) * 32],
                        rhs=cc[:, grp * 128:(grp + 1) * 128],
                        start=(grp == 0), stop=(grp == 3))
                nc.vector.tensor_copy(stage[b][:, blk * 128:(blk + 1) * 128], outp[:])

            # gate pipelines of block k-1 are interleaved between segment-sum
            # chunks of block k so ACT latency hides under chunk matmuls
            pending = []
            for blk in range(NBLK):
                ytb = pypool.tile([128, FEAT], f32, tag="ytb", name="ytb")
                for j in range(NCB):
                    c = blk * NCB + j
                    bi, sl = divmod(c, G)
                    while gather.nxt <= min(bi + 3, NCALLS - 1):
                        gather(gather.nxt)
                    gt = gts[bi]
                    S = spool.tile([128, 128], bf16, tag="S", name="S")
                    nc.vector.tensor_scalar(
                        S[:], iota_sb[:],
                        dstrel_sb[:, c:c + 1], norm_sb[:, c:c + 1],
                        Alu.is_equal, Alu.mult,
                    )
                    for q in range(4):
                        nc.tensor.matmul(
                            ytb[:, q * 512:(q + 1) * 512],
                            lhsT=S[:],
                            rhs=gt[:, sl * FEAT + q * 512: sl * FEAT + (q + 1) * 512],
                            start=(j == 0), stop=(j == NCB - 1),
                        )
                    if pending:
                        pending.pop(0)()
                while pending:
                    pending.pop(0)()

                ysb = ysbpool.tile([128, FEAT], bf16, tag="ysb", name="ysb")
                nc.vector.tensor_copy(ysb[:], ytb[:])
                ysbT = ytpool.tile([128, FEAT], bf16, tag="ysbT", name="ysbT")
                for q in range(4):
                    tp = ptpool.tile([128, 512], bf16, tag="tp", name="tp")
                    for k in range(4):
                        fs = q * 4 + k
                        nc.tensor.transpose(
                            tp[:, k * 128:(k + 1) * 128],
                            ysb[:, fs * 128:(fs + 1) * 128], ident_sb[:])
                    nc.vector.tensor_copy(ysbT[:, q * 512:(q + 1) * 512], tp[:])

                pending = [
                    (lambda blk=blk, b=b, t=ysbT: gate(blk, b, t))
                    for b in range(B)
                ]
            while pending:
                pending.pop(0)()

            for b in range(B):
                nc.sync.dma_start(out_d[b], stage[b][:])

    nc.compile()
    return nc


def kernel(**inputs):
    global LAST_RESULT
    shared, percore, unperm, NCB = prep_host(**inputs)
    nc = build_bass(NCB)
    in_maps = []
    for c in range(NCORES):
        m = dict(shared)
        m.update(percore[c])
        in_maps.append(m)
    res = run_bass_kernel_spmd(nc, in_maps, core_ids=list(range(NCORES)),
                               trace=os.environ.get("BASS_TRACE") == "1")
    LAST_RESULT = res
    out = np.empty((B, N, COUT), np.float32)
    core_of, col_of = unperm["core_of"], unperm["col_of"]
    for c in range(NCORES):
        r = res.results[c]["out"]  # (B, 32, NBLK*128)
        ids = np.where(core_of == c)[0]
        out[:, ids, :] = r[:, :, col_of[ids]].transpose(0, 2, 1)
    return out


# revision 24
# speedup vs baseline: 1.1670x; 1.0497x over previous
"""BA3TGCN2 Trainium2 kernel: dst-sharded GCN gather/segment-sum + GRU gate fusion.

Math (H0 == 0 makes the R gate dead and linearizes the layers):
  out[b,n,:] = sum_p ws[p] * sigmoid(-(Ahat x_p Uz + bz)) * tanh(Ahat x_p Uh + bh)
  Uz = Wcz @ Wlz[:COUT], bz = bcz @ Wlz[:COUT] + blz   (same for h with Wch/Wlh)
  ws = softmax(attention) (second half scaled by TRAIN_OR_PREDICT=1)

Sharding: dst nodes balanced-binned into 80 blocks of <=128 slots, 10 blocks per
core; every core handles ALL 16 batches for its ~1250 dst nodes (~E/8 edges).
Per-edge gather row: 4KB = 16 batches x 16 periods x 8 cin, bf16, fetched with
batched indirect DMA (G chunks / 640 rows per SWDGE call).
Per 128-edge chunk: S[e,d] = (iota==dstrel)*norm one-hot scatter matrix, then
ytb[d, feat] += S^T @ gathered (TensorE segment-sum, PSUM f32, 4 banks).
Per block: copy->SBUF bf16, 16 TensorE transposes -> per-batch (p*8+c, dst)
tiles, 8 gate matmuls vs fused U, sigmoid/tanh on ACT, product on DVE,
period-weighted sum via wsum matmul, stage, one DMA out per batch.
"""

import heapq
import os

import numpy as np
import ml_dtypes

import concourse.bass as bass
import concourse.bacc as bacc
from concourse._compat import get_trn_type
import concourse.mybir as mybir
import concourse.tile as tile
from concourse import library_config
from concourse.bass_utils import run_bass_kernel_spmd

BF16 = ml_dtypes.bfloat16

B, N, CIN, COUT, P2 = 16, 10000, 8, 32, 16
E = 160000
NCORES = 8
NBLK = 10                    # dst blocks per core, 128 slots each
NBINS = NCORES * NBLK        # 80
FEAT = B * P2 * CIN          # 2048 = full-batch feature row per node
G = 1                        # 128-edge chunks per indirect-gather call
TRAIN_OR_PREDICT = 1.0

LAST_RESULT = None           # BassKernelResults of last run (for test.py)


def _softmax(x):
    e = np.exp(x - np.max(x))
    return e / e.sum()


def prep_host(X, edge_index, edge_weight, attention,
              Wcz, bcz, Wlz, blz, Wcr, bcr, Wlr, blr, Wch, bch, Wlh, blh):
    X = np.asarray(X, np.float32)
    src = np.asarray(edge_index[0], np.int64)
    dst = np.asarray(edge_index[1], np.int64)
    w = np.asarray(edge_weight, np.float32)

    # gcn_norm with self loops
    loop = np.arange(N, dtype=np.int64)
    src = np.concatenate([src, loop])
    dst = np.concatenate([dst, loop])
    w = np.concatenate([w, np.ones(N, np.float32)])
    deg = np.bincount(dst, weights=w, minlength=N).astype(np.float32)
    dinv = np.where(deg > 0, deg.astype(np.float64) ** -0.5, 0.0).astype(np.float32)
    norm = (dinv[src] * w * dinv[dst]).astype(np.float32)

    # balance dst nodes into NBINS bins (<=128 each) with ~equal edge counts
    cnt = np.bincount(dst, minlength=N)
    order = np.argsort(-cnt, kind="stable")
    bin_of = np.empty(N, np.int32)
    slot_of = np.empty(N, np.int32)
    ndst = np.zeros(NBINS, np.int32)
    tot = np.zeros(NBINS, np.int64)
    hp = [(0, b) for b in range(NBINS)]
    heapq.heapify(hp)
    for nid in order:
        while True:
            t, b = heapq.heappop(hp)
            if ndst[b] < 128:
                break
        bin_of[nid] = b
        slot_of[nid] = ndst[b]
        ndst[b] += 1
        tot[b] += cnt[nid]
        if ndst[b] < 128:
            heapq.heappush(hp, (int(t + cnt[nid]), b))
    NCB = int(-(-int(tot.max()) // 128))     # chunks per block (uniform)
    NCH = NBLK * NCB

    # per-core edge streams: (core, 128 lanes, NCH cols), col = blk*NCB + chunk
    ebin = bin_of[dst]
    # sort by (bin, src): ascending src within each bin gives the 4KB gather
    # reads an ascending-address sweep over HBM (better row locality)
    eorder = np.lexsort((src, ebin))
    bcnt = np.bincount(ebin, minlength=NBINS)
    boff = np.concatenate([[0], np.cumsum(bcnt)])
    sb = ebin[eorder]
    pos = np.arange(sb.size) - boff[sb]
    lane = (pos % 128).astype(np.int64)
    col = (sb % NBLK).astype(np.int64) * NCB + pos // 128
    core = sb // NBLK
    gidx = np.zeros((NCORES, 128, NCH), np.int32)
    dstrel = np.zeros((NCORES, 128, NCH), np.float32)
    normt = np.zeros((NCORES, 128, NCH), np.float32)
    gidx[core, lane, col] = src[eorder]
    dstrel[core, lane, col] = slot_of[dst[eorder]]
    normt[core, lane, col] = norm[eorder]

    # dma_gather index layout: per call bi (G chunks), gathered row i=j*128+p
    # reads gidx[p, bi*G+j]; idxs wrap 16 partitions: idxs16[i%16, i//16]
    NCALLS = NCH // G
    CW = G * 128 // 16           # idx columns per call
    gidx16 = np.zeros((NCORES, 128, NCALLS * CW), np.int16)
    for bi in range(NCALLS):
        # unwrapped[i = j*128+p] = gidx[core, p, bi*G+j]
        unwrapped = np.ascontiguousarray(
            gidx[:, :, bi * G:(bi + 1) * G].transpose(0, 2, 1)  # (core, j, p)
        ).reshape(NCORES, G * 128).astype(np.int16)
        gidx16[:, :16, bi * CW:(bi + 1) * CW] = unwrapped.reshape(
            NCORES, CW, 16).transpose(0, 2, 1)

    # full-batch node feature table, row n = [b(16) x p(16) x cin(8)] bf16
    xtab = np.ascontiguousarray(
        X.transpose(1, 0, 3, 2).reshape(N, FEAT)).astype(BF16)

    # fused weights / biases / period weights
    Uz = np.asarray(Wcz, np.float32) @ np.asarray(Wlz, np.float32)[:COUT]
    Uh = np.asarray(Wch, np.float32) @ np.asarray(Wlh, np.float32)[:COUT]
    bz = np.asarray(bcz, np.float32) @ np.asarray(Wlz, np.float32)[:COUT] + np.asarray(blz, np.float32)
    bh = np.asarray(bch, np.float32) @ np.asarray(Wlh, np.float32)[:COUT] + np.asarray(blh, np.float32)
    probs = _softmax(np.asarray(attention, np.float32))
    ws = np.concatenate([probs[:P2 // 2], probs[P2 // 2:] * TRAIN_OR_PREDICT])

    # transform lhsT tiles: ubig[(p*8+cin), (g*4+grp)*128 + pl*32 + s] = (p==grp*4+pl)*U_g[cin,s]
    ubig = np.zeros((128, 2 * 4 * 128), np.float32)
    for g, U in enumerate((Uz, Uh)):
        for grp in range(4):
            for pl in range(4):
                p = grp * 4 + pl
                ubig[p * 8:(p + 1) * 8, (g * 4 + grp) * 128 + pl * 32:(g * 4 + grp) * 128 + (pl + 1) * 32] = U
    # weighted period-sum lhsT: wsum[(pl*32+s), grp*32+o] = ws[grp*4+pl]*(s==o)
    wsum = np.zeros((128, 4 * 32), np.float32)
    for grp in range(4):
        for pl in range(4):
            for s in range(32):
                wsum[pl * 32 + s, grp * 32 + s] = ws[grp * 4 + pl]
    biasz = np.repeat(-bz[None, :], 4, 0).reshape(128, 1).astype(np.float32)
    biash = np.repeat(bh[None, :], 4, 0).reshape(128, 1).astype(np.float32)

    iota = np.tile(np.arange(128, dtype=np.float32), (128, 1))
    ident = np.eye(128, dtype=np.float32)

    shared = dict(
        xtab=xtab,
        ubig=ubig.astype(BF16),
        wsum=wsum.astype(BF16),
        biasz=biasz,
        biash=biash,
        iota=iota.astype(BF16),
        ident=ident.astype(BF16),
    )
    percore = [dict(gidx=np.ascontiguousarray(gidx[c]),
                    dstrel=np.ascontiguousarray(dstrel[c]),
                    normt=np.ascontiguousarray(normt[c]))
               for c in range(NCORES)]
    unperm = dict(core_of=bin_of // NBLK,
                  col_of=(bin_of % NBLK) * 128 + slot_of)
    return shared, percore, unperm, NCB


def build_bass(NCB):
    NCH = NBLK * NCB
    f32 = mybir.dt.float32
    bf16 = mybir.dt.bfloat16
    i32 = mybir.dt.int32
    Alu = mybir.AluOpType
    Act = mybir.ActivationFunctionType
    NCALLS = NCH // G
    CW = G * 128 // 16

    nc = bacc.Bacc(get_trn_type() or "TRN2")
    xtab_d = nc.dram_tensor("xtab", (N, FEAT), bf16, kind="ExternalInput")
    gidx_d = nc.dram_tensor("gidx", (128, NCH), i32, kind="ExternalInput")
    dstrel_d = nc.dram_tensor("dstrel", (128, NCH), f32, kind="ExternalInput")
    normt_d = nc.dram_tensor("normt", (128, NCH), f32, kind="ExternalInput")
    ubig_d = nc.dram_tensor("ubig", (128, 1024), bf16, kind="ExternalInput")
    wsum_d = nc.dram_tensor("wsum", (128, 128), bf16, kind="ExternalInput")
    biasz_d = nc.dram_tensor("biasz", (128, 1), f32, kind="ExternalInput")
    biash_d = nc.dram_tensor("biash", (128, 1), f32, kind="ExternalInput")
    iota_d = nc.dram_tensor("iota", (128, 128), bf16, kind="ExternalInput")
    ident_d = nc.dram_tensor("ident", (128, 128), bf16, kind="ExternalInput")
    out_d = nc.dram_tensor("out", (B, 32, NBLK * 128), f32, kind="ExternalOutput")

    with tile.TileContext(nc) as tc:
        with tc.tile_pool(name="const", bufs=1) as cpool, \
             tc.tile_pool(name="gp", bufs=6) as gpool, \
             tc.tile_pool(name="sp", bufs=8) as spool, \
             tc.tile_pool(name="ysb", bufs=2) as ysbpool, \
             tc.tile_pool(name="yt", bufs=2) as ytpool, \
             tc.tile_pool(name="wk", bufs=2) as wkpool, \
             tc.tile_pool(name="st", bufs=1) as stpool, \
             tc.tile_pool(name="py", bufs=1, space="PSUM") as pypool, \
             tc.tile_pool(name="pt", bufs=1, space="PSUM") as ptpool, \
             tc.tile_pool(name="pg", bufs=1, space="PSUM") as pgpool, \
             tc.tile_pool(name="po", bufs=1, space="PSUM") as popool:

            def cload(dram, shape, dtype, name):
                t = cpool.tile(shape, dtype, name=name, tag=name)
                nc.sync.dma_start(t[:], dram[:])
                return t

            gidx_sb = cload(gidx_d, [128, NCH], i32, "gidx_sb")
            dstrel_sb = cload(dstrel_d, [128, NCH], f32, "dstrel_sb")
            norm_sb = cload(normt_d, [128, NCH], f32, "norm_sb")
            ubig_sb = cload(ubig_d, [128, 1024], bf16, "ubig_sb")
            wsum_sb = cload(wsum_d, [128, 128], bf16, "wsum_sb")
            biasz_sb = cload(biasz_d, [128, 1], f32, "biasz_sb")
            biash_sb = cload(biash_d, [128, 1], f32, "biash_sb")
            iota_sb = cload(iota_d, [128, 128], bf16, "iota_sb")
            ident_sb = cload(ident_d, [128, 128], bf16, "ident_sb")

            stage = [stpool.tile([32, NBLK * 128], f32, name=f"stage{b}", tag=f"stage{b}")
                     for b in range(B)]

            gts = {}

            def gather(bi):
                gt = gpool.tile([128, G * FEAT], bf16, tag="gt", name="gt")
                nc.gpsimd.indirect_dma_start(
                    out=gt[:],
                    out_offset=None,
                    in_=xtab_d[:, :],
                    in_offset=bass.IndirectOffsetOnAxis(
                        ap=gidx_sb[:, bi * G:(bi + 1) * G], axis=0),
                )
                gts[bi] = gt
                gather.nxt = bi + 1
            gather.nxt = 0

            def gate(blk, b, ysbT):
                rhs = ysbT[:, b * 128:(b + 1) * 128]
                az = pgpool.tile([128, 512], f32, tag="az", name="az")
                ah = pgpool.tile([128, 512], f32, tag="ah", name="ah")
                for grp in range(4):
                    nc.tensor.matmul(
                        az[:, grp * 128:(grp + 1) * 128],
                        lhsT=ubig_sb[:, grp * 128:(grp + 1) * 128],
                        rhs=rhs, start=True, stop=True)
                    nc.tensor.matmul(
                        ah[:, grp * 128:(grp + 1) * 128],
                        lhsT=ubig_sb[:, (4 + grp) * 128:(5 + grp) * 128],
                        rhs=rhs, start=True, stop=True)
                zp = wkpool.tile([128, 512], bf16, tag="zp", name="zp")
                tp2 = wkpool.tile([128, 512], bf16, tag="tp2", name="tp2")
                nc.scalar.activation(zp[:], az[:], Act.Sigmoid,
                                     bias=biasz_sb[:, :1], scale=-1.0)
                nc.scalar.activation(tp2[:], ah[:], Act.Tanh,
                                     bias=biash_sb[:, :1], scale=1.0)
                cc = wkpool.tile([128, 512], bf16, tag="cc", name="cc")
                nc.vector.tensor_tensor(cc[:], zp[:], tp2[:], op=Alu.mult)
                outp = popool.tile([32, 128], f32, tag="outp", name="outp")
                for grp in range(4):
                    nc.tensor.matmul(
                        outp[:],
                        lhsT=wsum_sb[:, grp * 32:(grp + 1) * 32],
                        rhs=cc[:, grp * 128:(grp + 1) * 128],
                        start=(grp == 0), stop=(grp == 3))
                nc.vector.tensor_copy(stage[b][:, blk * 128:(blk + 1) * 128], outp[:])

            # gate pipelines of block k-1 are interleaved between segment-sum
            # chunks of block k so ACT latency hides under chunk matmuls
            pending = []
            for blk in range(NBLK):
                ytb = pypool.tile([128, FEAT], f32, tag="ytb", name="ytb")
                for j in range(NCB):
                    c = blk * NCB + j
                    bi, sl = divmod(c, G)
                    while gather.nxt <= min(bi + 5, NCALLS - 1):
                        gather(gather.nxt)
                    gt = gts[bi]
                    S = spool.tile([128, 128], bf16, tag="S", name="S")
                    nc.vector.tensor_scalar(
                        S[:], iota_sb[:],
                        dstrel_sb[:, c:c + 1], norm_sb[:, c:c + 1],
                        Alu.is_equal, Alu.mult,
                    )
                    for q in range(4):
                        nc.tensor.matmul(
                            ytb[:, q * 512:(q + 1) * 512],
                            lhsT=S[:],
                            rhs=gt[:, sl * FEAT + q * 512: sl * FEAT + (q + 1) * 512],
                            start=(j == 0), stop=(j == NCB - 1),
                        )
                    if pending:
                        pending.pop(0)()
                while pending:
                    pending.pop(0)()

                ysb = ysbpool.tile([128, FEAT], bf16, tag="ysb", name="ysb")
                nc.vector.tensor_copy(ysb[:], ytb[:])
                ysbT = ytpool.tile([128, FEAT], bf16, tag="ysbT", name="ysbT")
                for q in range(4):
                    tp = ptpool.tile([128, 512], bf16, tag="tp", name="tp")
                    for k in range(4):
                        fs = q * 4 + k
                        nc.tensor.transpose(
                            tp[:, k * 128:(k + 1) * 128],
                            ysb[:, fs * 128:(fs + 1) * 128], ident_sb[:])
                    nc.vector.tensor_copy(ysbT[:, q * 512:(q + 1) * 512], tp[:])

                pending = [
                    (lambda blk=blk, b=b, t=ysbT: gate(blk, b, t))
                    for b in range(B)
                ]
            while pending:
                pending.pop(0)()

            for b in range(B):
                nc.sync.dma_start(out_d[b], stage[b][:])

    nc.compile()
    return nc


def kernel(**inputs):
    global LAST_RESULT
    shared, percore, unperm, NCB = prep_host(**inputs)
    nc = build_bass(NCB)
    in_maps = []
    for c in range(NCORES):
        m = dict(shared)
        m.update(percore[c])
        in_maps.append(m)
    res = run_bass_kernel_spmd(nc, in_maps, core_ids=list(range(NCORES)),
                               trace=os.environ.get("BASS_TRACE") == "1")
    LAST_RESULT = res
    out = np.empty((B, N, COUT), np.float32)
    core_of, col_of = unperm["core_of"], unperm["col_of"]
    for c in range(NCORES):
        r = res.results[c]["out"]  # (B, 32, NBLK*128)
        ids = np.where(core_of == c)[0]
        out[:, ids, :] = r[:, :, col_of[ids]].transpose(0, 2, 1)
    return out


# revision 25
# speedup vs baseline: 1.1925x; 1.0218x over previous
"""BA3TGCN2 Trainium2 kernel: dst-sharded GCN gather/segment-sum + GRU gate fusion.

Math (H0 == 0 makes the R gate dead and linearizes the layers):
  out[b,n,:] = sum_p ws[p] * sigmoid(-(Ahat x_p Uz + bz)) * tanh(Ahat x_p Uh + bh)
  Uz = Wcz @ Wlz[:COUT], bz = bcz @ Wlz[:COUT] + blz   (same for h with Wch/Wlh)
  ws = softmax(attention) (second half scaled by TRAIN_OR_PREDICT=1)

Sharding: dst nodes balanced-binned into 80 blocks of <=128 slots, 10 blocks per
core; every core handles ALL 16 batches for its ~1250 dst nodes (~E/8 edges).
Per-edge gather row: 4KB = 16 batches x 16 periods x 8 cin, bf16, fetched with
batched indirect DMA (G chunks / 640 rows per SWDGE call).
Per 128-edge chunk: S[e,d] = (iota==dstrel)*norm one-hot scatter matrix, then
ytb[d, feat] += S^T @ gathered (TensorE segment-sum, PSUM f32, 4 banks).
Per block: copy->SBUF bf16, 16 TensorE transposes -> per-batch (p*8+c, dst)
tiles, 8 gate matmuls vs fused U, sigmoid/tanh on ACT, product on DVE,
period-weighted sum via wsum matmul, stage, one DMA out per batch.
"""

import heapq
import os

import numpy as np
import ml_dtypes

import concourse.bass as bass
import concourse.bacc as bacc
from concourse._compat import get_trn_type
import concourse.mybir as mybir
import concourse.tile as tile
from concourse import library_config
from concourse.bass_utils import run_bass_kernel_spmd

BF16 = ml_dtypes.bfloat16
FP8E3 = ml_dtypes.float8_e3m4

B, N, CIN, COUT, P2 = 16, 10000, 8, 32, 16
E = 160000
NCORES = 8
NBLK = 10                    # dst blocks per core, 128 slots each
NBINS = NCORES * NBLK        # 80
FEAT = B * P2 * CIN          # 2048 = full-batch feature row per node
G = 1                        # 128-edge chunks per indirect-gather call
TRAIN_OR_PREDICT = 1.0

LAST_RESULT = None           # BassKernelResults of last run (for test.py)


def _softmax(x):
    e = np.exp(x - np.max(x))
    return e / e.sum()


def prep_host(X, edge_index, edge_weight, attention,
              Wcz, bcz, Wlz, blz, Wcr, bcr, Wlr, blr, Wch, bch, Wlh, blh):
    X = np.asarray(X, np.float32)
    src = np.asarray(edge_index[0], np.int64)
    dst = np.asarray(edge_index[1], np.int64)
    w = np.asarray(edge_weight, np.float32)

    # gcn_norm with self loops
    loop = np.arange(N, dtype=np.int64)
    src = np.concatenate([src, loop])
    dst = np.concatenate([dst, loop])
    w = np.concatenate([w, np.ones(N, np.float32)])
    deg = np.bincount(dst, weights=w, minlength=N).astype(np.float32)
    dinv = np.where(deg > 0, deg.astype(np.float64) ** -0.5, 0.0).astype(np.float32)
    norm = (dinv[src] * w * dinv[dst]).astype(np.float32)

    # balance dst nodes into NBINS bins (<=128 each) with ~equal edge counts
    cnt = np.bincount(dst, minlength=N)
    order = np.argsort(-cnt, kind="stable")
    bin_of = np.empty(N, np.int32)
    slot_of = np.empty(N, np.int32)
    ndst = np.zeros(NBINS, np.int32)
    tot = np.zeros(NBINS, np.int64)
    hp = [(0, b) for b in range(NBINS)]
    heapq.heapify(hp)
    for nid in order:
        while True:
            t, b = heapq.heappop(hp)
            if ndst[b] < 128:
                break
        bin_of[nid] = b
        slot_of[nid] = ndst[b]
        ndst[b] += 1
        tot[b] += cnt[nid]
        if ndst[b] < 128:
            heapq.heappush(hp, (int(t + cnt[nid]), b))
    NCB = int(-(-int(tot.max()) // 128))     # chunks per block (uniform)
    NCH = NBLK * NCB

    # per-core edge streams: (core, 128 lanes, NCH cols), col = blk*NCB + chunk
    ebin = bin_of[dst]
    # sort by (bin, src): ascending src within each bin gives the 4KB gather
    # reads an ascending-address sweep over HBM (better row locality)
    eorder = np.lexsort((src, ebin))
    bcnt = np.bincount(ebin, minlength=NBINS)
    boff = np.concatenate([[0], np.cumsum(bcnt)])
    sb = ebin[eorder]
    pos = np.arange(sb.size) - boff[sb]
    lane = (pos % 128).astype(np.int64)
    col = (sb % NBLK).astype(np.int64) * NCB + pos // 128
    core = sb // NBLK
    # full-batch node feature table, row n = [b(16) x p(16) x cin(8)] fp8-e3m4
    # (cast to bf16 in-flight by the gather DMA); per-node scaled into the
    # e3m4 sweet spot, 1/scale folded into the edge norms
    xt32 = np.ascontiguousarray(X.transpose(1, 0, 3, 2).reshape(N, FEAT))
    xscale = (12.0 / np.maximum(np.abs(xt32).max(axis=1), 1e-9)).astype(np.float32)
    xtab = (xt32 * xscale[:, None]).astype(FP8E3)

    gidx = np.zeros((NCORES, 128, NCH), np.int32)
    dstrel = np.zeros((NCORES, 128, NCH), np.float32)
    normt = np.zeros((NCORES, 128, NCH), np.float32)
    gidx[core, lane, col] = src[eorder]
    dstrel[core, lane, col] = slot_of[dst[eorder]]
    normt[core, lane, col] = norm[eorder] / xscale[src[eorder]]

    # dma_gather index layout: per call bi (G chunks), gathered row i=j*128+p
    # reads gidx[p, bi*G+j]; idxs wrap 16 partitions: idxs16[i%16, i//16]
    NCALLS = NCH // G
    CW = G * 128 // 16           # idx columns per call
    gidx16 = np.zeros((NCORES, 128, NCALLS * CW), np.int16)
    for bi in range(NCALLS):
        # unwrapped[i = j*128+p] = gidx[core, p, bi*G+j]
        unwrapped = np.ascontiguousarray(
            gidx[:, :, bi * G:(bi + 1) * G].transpose(0, 2, 1)  # (core, j, p)
        ).reshape(NCORES, G * 128).astype(np.int16)
        gidx16[:, :16, bi * CW:(bi + 1) * CW] = unwrapped.reshape(
            NCORES, CW, 16).transpose(0, 2, 1)


    # fused weights / biases / period weights
    Uz = np.asarray(Wcz, np.float32) @ np.asarray(Wlz, np.float32)[:COUT]
    Uh = np.asarray(Wch, np.float32) @ np.asarray(Wlh, np.float32)[:COUT]
    bz = np.asarray(bcz, np.float32) @ np.asarray(Wlz, np.float32)[:COUT] + np.asarray(blz, np.float32)
    bh = np.asarray(bch, np.float32) @ np.asarray(Wlh, np.float32)[:COUT] + np.asarray(blh, np.float32)
    probs = _softmax(np.asarray(attention, np.float32))
    ws = np.concatenate([probs[:P2 // 2], probs[P2 // 2:] * TRAIN_OR_PREDICT])

    # transform lhsT tiles: ubig[(p*8+cin), (g*4+grp)*128 + pl*32 + s] = (p==grp*4+pl)*U_g[cin,s]
    ubig = np.zeros((128, 2 * 4 * 128), np.float32)
    for g, U in enumerate((Uz, Uh)):
        for grp in range(4):
            for pl in range(4):
                p = grp * 4 + pl
                ubig[p * 8:(p + 1) * 8, (g * 4 + grp) * 128 + pl * 32:(g * 4 + grp) * 128 + (pl + 1) * 32] = U
    # weighted period-sum lhsT: wsum[(pl*32+s), grp*32+o] = ws[grp*4+pl]*(s==o)
    wsum = np.zeros((128, 4 * 32), np.float32)
    for grp in range(4):
        for pl in range(4):
            for s in range(32):
                wsum[pl * 32 + s, grp * 32 + s] = ws[grp * 4 + pl]
    biasz = np.repeat(-bz[None, :], 4, 0).reshape(128, 1).astype(np.float32)
    biash = np.repeat(bh[None, :], 4, 0).reshape(128, 1).astype(np.float32)

    iota = np.tile(np.arange(128, dtype=np.float32), (128, 1))
    ident = np.eye(128, dtype=np.float32)

    shared = dict(
        xtab=xtab,
        ubig=ubig.astype(BF16),
        wsum=wsum.astype(BF16),
        biasz=biasz,
        biash=biash,
        iota=iota.astype(BF16),
        ident=ident.astype(BF16),
    )
    percore = [dict(gidx=np.ascontiguousarray(gidx[c]),
                    dstrel=np.ascontiguousarray(dstrel[c]),
                    normt=np.ascontiguousarray(normt[c]))
               for c in range(NCORES)]
    unperm = dict(core_of=bin_of // NBLK,
                  col_of=(bin_of % NBLK) * 128 + slot_of)
    return shared, percore, unperm, NCB


def build_bass(NCB):
    NCH = NBLK * NCB
    f32 = mybir.dt.float32
    bf16 = mybir.dt.bfloat16
    i32 = mybir.dt.int32
    fp8e3 = mybir.dt.float8e3
    Alu = mybir.AluOpType
    Act = mybir.ActivationFunctionType
    NCALLS = NCH // G
    CW = G * 128 // 16

    nc = bacc.Bacc(get_trn_type() or "TRN2")
    xtab_d = nc.dram_tensor("xtab", (N, FEAT), fp8e3, kind="ExternalInput")
    gidx_d = nc.dram_tensor("gidx", (128, NCH), i32, kind="ExternalInput")
    dstrel_d = nc.dram_tensor("dstrel", (128, NCH), f32, kind="ExternalInput")
    normt_d = nc.dram_tensor("normt", (128, NCH), f32, kind="ExternalInput")
    ubig_d = nc.dram_tensor("ubig", (128, 1024), bf16, kind="ExternalInput")
    wsum_d = nc.dram_tensor("wsum", (128, 128), bf16, kind="ExternalInput")
    biasz_d = nc.dram_tensor("biasz", (128, 1), f32, kind="ExternalInput")
    biash_d = nc.dram_tensor("biash", (128, 1), f32, kind="ExternalInput")
    iota_d = nc.dram_tensor("iota", (128, 128), bf16, kind="ExternalInput")
    ident_d = nc.dram_tensor("ident", (128, 128), bf16, kind="ExternalInput")
    out_d = nc.dram_tensor("out", (B, 32, NBLK * 128), f32, kind="ExternalOutput")

    with tile.TileContext(nc) as tc:
        with tc.tile_pool(name="const", bufs=1) as cpool, \
             tc.tile_pool(name="gp", bufs=6) as gpool, \
             tc.tile_pool(name="sp", bufs=8) as spool, \
             tc.tile_pool(name="ysb", bufs=2) as ysbpool, \
             tc.tile_pool(name="yt", bufs=2) as ytpool, \
             tc.tile_pool(name="wk", bufs=2) as wkpool, \
             tc.tile_pool(name="st", bufs=1) as stpool, \
             tc.tile_pool(name="py", bufs=1, space="PSUM") as pypool, \
             tc.tile_pool(name="pt", bufs=1, space="PSUM") as ptpool, \
             tc.tile_pool(name="pg", bufs=1, space="PSUM") as pgpool, \
             tc.tile_pool(name="po", bufs=1, space="PSUM") as popool:

            def cload(dram, shape, dtype, name):
                t = cpool.tile(shape, dtype, name=name, tag=name)
                nc.sync.dma_start(t[:], dram[:])
                return t

            gidx_sb = cload(gidx_d, [128, NCH], i32, "gidx_sb")
            dstrel_sb = cload(dstrel_d, [128, NCH], f32, "dstrel_sb")
            norm_sb = cload(normt_d, [128, NCH], f32, "norm_sb")
            ubig_sb = cload(ubig_d, [128, 1024], bf16, "ubig_sb")
            wsum_sb = cload(wsum_d, [128, 128], bf16, "wsum_sb")
            biasz_sb = cload(biasz_d, [128, 1], f32, "biasz_sb")
            biash_sb = cload(biash_d, [128, 1], f32, "biash_sb")
            iota_sb = cload(iota_d, [128, 128], bf16, "iota_sb")
            ident_sb = cload(ident_d, [128, 128], bf16, "ident_sb")

            stage = [stpool.tile([32, NBLK * 128], f32, name=f"stage{b}", tag=f"stage{b}")
                     for b in range(B)]

            gts = {}

            def gather(bi):
                gt = gpool.tile([128, G * FEAT], bf16, tag="gt", name="gt")
                nc.gpsimd.indirect_dma_start(
                    out=gt[:],
                    out_offset=None,
                    in_=xtab_d[:, :],
                    in_offset=bass.IndirectOffsetOnAxis(
                        ap=gidx_sb[:, bi * G:(bi + 1) * G], axis=0),
                )
                gts[bi] = gt
                gather.nxt = bi + 1
            gather.nxt = 0

            def gate(blk, b, ysbT):
                rhs = ysbT[:, b * 128:(b + 1) * 128]
                az = pgpool.tile([128, 512], f32, tag="az", name="az")
                ah = pgpool.tile([128, 512], f32, tag="ah", name="ah")
                for grp in range(4):
                    nc.tensor.matmul(
                        az[:, grp * 128:(grp + 1) * 128],
                        lhsT=ubig_sb[:, grp * 128:(grp + 1) * 128],
                        rhs=rhs, start=True, stop=True)
                    nc.tensor.matmul(
                        ah[:, grp * 128:(grp + 1) * 128],
                        lhsT=ubig_sb[:, (4 + grp) * 128:(5 + grp) * 128],
                        rhs=rhs, start=True, stop=True)
                zp = wkpool.tile([128, 512], bf16, tag="zp", name="zp")
                tp2 = wkpool.tile([128, 512], bf16, tag="tp2", name="tp2")
                nc.scalar.activation(zp[:], az[:], Act.Sigmoid,
                                     bias=biasz_sb[:, :1], scale=-1.0)
                nc.scalar.activation(tp2[:], ah[:], Act.Tanh,
                                     bias=biash_sb[:, :1], scale=1.0)
                cc = wkpool.tile([128, 512], bf16, tag="cc", name="cc")
                nc.vector.tensor_tensor(cc[:], zp[:], tp2[:], op=Alu.mult)
                outp = popool.tile([32, 128], f32, tag="outp", name="outp")
                for grp in range(4):
                    nc.tensor.matmul(
                        outp[:],
                        lhsT=wsum_sb[:, grp * 32:(grp + 1) * 32],
                        rhs=cc[:, grp * 128:(grp + 1) * 128],
                        start=(grp == 0), stop=(grp == 3))
                nc.vector.tensor_copy(stage[b][:, blk * 128:(blk + 1) * 128], outp[:])

            # gate pipelines of block k-1 are interleaved between segment-sum
            # chunks of block k so ACT latency hides under chunk matmuls
            pending = []
            for blk in range(NBLK):
                ytb = pypool.tile([128, FEAT], f32, tag="ytb", name="ytb")
                for j in range(NCB):
                    c = blk * NCB + j
                    bi, sl = divmod(c, G)
                    while gather.nxt <= min(bi + 5, NCALLS - 1):
                        gather(gather.nxt)
                    gt = gts[bi]
                    S = spool.tile([128, 128], bf16, tag="S", name="S")
                    nc.vector.tensor_scalar(
                        S[:], iota_sb[:],
                        dstrel_sb[:, c:c + 1], norm_sb[:, c:c + 1],
                        Alu.is_equal, Alu.mult,
                    )
                    for q in range(4):
                        nc.tensor.matmul(
                            ytb[:, q * 512:(q + 1) * 512],
                            lhsT=S[:],
                            rhs=gt[:, sl * FEAT + q * 512: sl * FEAT + (q + 1) * 512],
                            start=(j == 0), stop=(j == NCB - 1),
                        )
                    if pending:
                        pending.pop(0)()
                while pending:
                    pending.pop(0)()

                ysb = ysbpool.tile([128, FEAT], bf16, tag="ysb", name="ysb")
                nc.vector.tensor_copy(ysb[:], ytb[:])
                ysbT = ytpool.tile([128, FEAT], bf16, tag="ysbT", name="ysbT")
                for q in range(4):
                    tp = ptpool.tile([128, 512], bf16, tag="tp", name="tp")
                    for k in range(4):
                        fs = q * 4 + k
                        nc.tensor.transpose(
                            tp[:, k * 128:(k + 1) * 128],
                            ysb[:, fs * 128:(fs + 1) * 128], ident_sb[:])
                    nc.vector.tensor_copy(ysbT[:, q * 512:(q + 1) * 512], tp[:])

                pending = [
                    (lambda blk=blk, b=b, t=ysbT: gate(blk, b, t))
                    for b in range(B)
                ]
            while pending:
                pending.pop(0)()

            for b in range(B):
                nc.sync.dma_start(out_d[b], stage[b][:])

    nc.compile()
    return nc


def kernel(**inputs):
    global LAST_RESULT
    shared, percore, unperm, NCB = prep_host(**inputs)
    nc = build_bass(NCB)
    in_maps = []
    for c in range(NCORES):
        m = dict(shared)
        m.update(percore[c])
        in_maps.append(m)
    res = run_bass_kernel_spmd(nc, in_maps, core_ids=list(range(NCORES)),
                               trace=os.environ.get("BASS_TRACE") == "1")
    LAST_RESULT = res
    out = np.empty((B, N, COUT), np.float32)
    core_of, col_of = unperm["core_of"], unperm["col_of"]
    for c in range(NCORES):
        r = res.results[c]["out"]  # (B, 32, NBLK*128)
        ids = np.where(core_of == c)[0]
        out[:, ids, :] = r[:, :, col_of[ids]].transpose(0, 2, 1)
    return out
